# revision 15
# baseline (speedup 1.0000x reference)
"""MinGRU Trainium2 Bass kernel.

Problem: x (128, 1024, 256) f32, Wz/Wh (768, 512) f32, bz/bh (512,) f32.
    xz = x @ Wz[:256] + bz ; xh = x @ Wh[:256] + bh   (precomputable)
    per step t: z = sigmoid(xz_t + h @ Wz[256:])
                g = tanh   (xh_t + h @ Wh[256:])
                h = (1-z)*h + z*g
Returns (outputs (128, 1024, 512) f32, h_last (128, 512) f32).

Strategy: data-parallel over batch across 8 NeuronCores (16 rows each).
Per core everything lives in "T orientation" (hidden dim on the 128
partitions, batch on the free dim) so the per-step elementwise tail is
cheap ([128, 64] ops) and no transposes are ever needed:

  - recurrent matmuls: stationary = bf16 weight tile W[k,m] (128x128),
    moving = h^T k-slice (128x16), PSUM accumulates a^T m-slice (128x16).
    2 gates x 4 m x 4 k = 32 self-loading matmuls per step.
  - xz/xh are precomputed per 128-step chunk by a GEMM whose moving
    operand is x^T; the precompute matmuls are spread between step
    matmuls so the PE pipeline gaps at step boundaries absorb them.
  - outputs are stored as outT (T, 4, 128, 16) and untransposed on host.

bf16 is used for matmul operands only (weights, h, xz/xh); PSUM
accumulation and the gate arithmetic stay fp32.  Measured error vs the
fp32 reference on hardware: ~3.8e-3 rel l2 / ~7.7e-3 absmax.
"""

import numpy as np

B_TOT, T, I, H = 128, 1024, 256, 512
NCORES = 8
B = B_TOT // NCORES          # 16 batch rows per core
KH = H // 128                # 4 hidden contraction chunks
KI = I // 128                # 2 input contraction chunks
M = H // 128                 # 4 output-hidden chunks
TC = 128                     # time steps per xz/xh precompute chunk
NCH = T // TC                # 8 chunks
TG = 16                      # steps per output staging/DMA group


def build(nc, T=T, TC=TC, TG=TG, reps=1):
    """Emit the full per-core kernel IR into `nc`. Reads DRAM params
    x/Wz/bz/Wh/bh, writes outT. reps>1 repeats the whole pass (identical
    output) -- used only to delta-time one pass on hardware."""
    import concourse.tile as tile
    from concourse import mybir
    from contextlib import ExitStack

    dt = mybir.dt
    AF = mybir.ActivationFunctionType
    NCH = T // TC

    x_d = nc.declare_dram_parameter("x", [B, T, I], dt.float32, isOutput=False)
    Wz_d = nc.declare_dram_parameter("Wz", [I + H, H], dt.float32, isOutput=False)
    bz_d = nc.declare_dram_parameter("bz", [H], dt.float32, isOutput=False)
    Wh_d = nc.declare_dram_parameter("Wh", [I + H, H], dt.float32, isOutput=False)
    bh_d = nc.declare_dram_parameter("bh", [H], dt.float32, isOutput=False)
    outT_d = nc.declare_dram_parameter("outT", [T, M, 128, B], dt.float32, isOutput=True)

    W_d = [Wz_d, Wh_d]
    b_d = [bz_d, bh_d]

    with tile.TileContext(nc) as tc, ExitStack() as ctx:
        const = ctx.enter_context(tc.tile_pool(name="const", bufs=1))
        wtmp_p = ctx.enter_context(tc.tile_pool(name="wtmp", bufs=2))
        xf_p = ctx.enter_context(tc.tile_pool(name="xf", bufs=2))
        xbf_p = ctx.enter_context(tc.tile_pool(name="xbf", bufs=2))
        xzt_p = ctx.enter_context(tc.tile_pool(name="xzt", bufs=2))
        hbf_p = ctx.enter_context(tc.tile_pool(name="hbf", bufs=3))
        sm_p = ctx.enter_context(tc.tile_pool(name="sm", bufs=3))
        stage_p = ctx.enter_context(tc.tile_pool(name="stage", bufs=2))
        ps_step = ctx.enter_context(tc.tile_pool(name="ps_step", bufs=3, space="PSUM"))
        ps_pre = ctx.enter_context(tc.tile_pool(name="ps_pre", bufs=2, space="PSUM"))

        # ---- one-time weight prep: fp32 DRAM -> bf16 SBUF tiles ----
        # recurrent weights: Wg_bf[:, g, k, m, :] is the 128x128 stationary
        # tile for out chunk m, contraction chunk k of gate g.
        Wg_bf = const.tile([128, 2, KH, M, 128], dt.bfloat16, name="Wg_bf")
        # projection weights: Wx_bf[:, g, k, m, :]
        Wx_bf = const.tile([128, 2, KI, M, 128], dt.bfloat16, name="Wx_bf")
        for g in range(2):
            for k in range(KH):
                wt = wtmp_p.tile([128, M, 128], dt.float32, tag="wtmp", name="wt")
                nc.gpsimd.dma_start(
                    out=wt,
                    in_=W_d[g][I + 128 * k : I + 128 * (k + 1), :].rearrange(
                        "p (m c) -> p m c", c=128
                    ),
                )
                nc.gpsimd.tensor_copy(Wg_bf[:, g, k], wt)
            for k in range(KI):
                wt = wtmp_p.tile([128, M, 128], dt.float32, tag="wtmp", name="wt")
                nc.gpsimd.dma_start(
                    out=wt,
                    in_=W_d[g][128 * k : 128 * (k + 1), :].rearrange(
                        "p (m c) -> p m c", c=128
                    ),
                )
                nc.gpsimd.tensor_copy(Wx_bf[:, g, k], wt)
        # biases: bias_sb[:, g*M + m] = b_g[128*m + p]
        bias_sb = const.tile([128, 2 * M], dt.float32, name="bias_sb")
        for g in range(2):
            nc.gpsimd.dma_start(
                out=bias_sb[:, g * M : (g + 1) * M],
                in_=b_d[g].rearrange("(m p) -> p m", p=128),
            )

        # ---- helpers ----
        def emit_x_load(c):
            """DMA chunk c of x (transposed, b-major) and cast to bf16."""
            xf = xf_p.tile([128, KI, B, TC], dt.float32, tag="xf", name="xf")
            xbf = xbf_p.tile([128, KI, B, TC], dt.bfloat16, tag="xbf", name="xbf")
            for k in range(KI):
                for b in range(B):
                    nc.sync.dma_start(
                        out=xf[:, k, b],
                        in_=x_d[b, c * TC : (c + 1) * TC, 128 * k : 128 * (k + 1)]
                        .rearrange("t p -> p t"),
                    )
            nc.gpsimd.tensor_copy(xbf, xf)
            return xbf

        NSUB = (TC * B) // 512      # 512-column sub-tiles of the precompute GEMM
        TSUB = 512 // B             # time steps covered by one sub-tile

        def make_xzt(c):
            return xzt_p.tile([128, 2, TC, M, B], dt.bfloat16, tag="xzt", name="xzt")

        def emit_pre_group(xbf, xzt, g, m, n):
            """One precompute group: xzT[g, ., m, .] for steps of sub-tile n."""
            pp = ps_pre.tile([128, B, TSUB], dt.float32, tag="pp", name="pp")
            for k in range(KI):
                nc.tensor.matmul(
                    pp,
                    lhsT=Wx_bf[:, g, k, m],
                    rhs=xbf[:, k, :, n * TSUB : (n + 1) * TSUB],
                    start=(k == 0),
                    stop=(k == KI - 1),
                )
            nc.scalar.activation(
                xzt[:, g, n * TSUB : (n + 1) * TSUB, m].rearrange("p t b -> p b t"),
                pp,
                AF.Identity,
                bias=bias_sb[:, g * M + m : g * M + m + 1],
            )

        def one_pass():
            # ---- initial state ----
            h_cur = hbf_p.tile([128, KH, B], dt.bfloat16, tag="h", name="h0")
            nc.vector.memset(h_cur, 0.0)

            xbf_cur = emit_x_load(0)
            xzt_cur = make_xzt(0)
            for g in range(2):
                for m in range(M):
                    for n in range(NSUB):
                        emit_pre_group(xbf_cur, xzt_cur, g, m, n)

            # ---- main loop over chunks and steps ----
            xbf_next = xzt_next = None
            for c in range(NCH):
                # prefetch next chunk's x, allocate next xzT
                if c + 1 < NCH:
                    xbf_next = emit_x_load(c + 1)
                    xzt_next = make_xzt(c + 1)
                    pre_groups = [
                        (g, m, n)
                        for g in range(2)
                        for m in range(M)
                        for n in range(NSUB)
                    ]
                    # spread precompute groups across this chunk's steps,
                    # starting a few steps in so the x DMA+cast can land
                    period = max(1, (TC - 8) // len(pre_groups))
                    pre_sched = {8 + i * period for i in range(len(pre_groups))}
                else:
                    pre_groups = []
                    pre_sched = set()

                stage = None
                h_new = None
                for t in range(TC):
                    s = c * TC + t
                    tau = t % TG
                    if tau == 0:
                        stage = stage_p.tile(
                            [128, TG, M, B], dt.float32, tag="stage", name="stage"
                        )

                    # interleave one precompute group for chunk c+1
                    if pre_groups and t in pre_sched:
                        g_, m_, n_ = pre_groups.pop(0)
                        emit_pre_group(xbf_next, xzt_next, g_, m_, n_)

                    psum = [
                        ps_step.tile([128, M, B], dt.float32, tag="ps_z", name="ps_z"),
                        ps_step.tile([128, M, B], dt.float32, tag="ps_h", name="ps_h"),
                    ]
                    # gate 1 (tanh / candidate) first, gate 0 (sigmoid) last:
                    # the tanh chain overlaps the z matmuls.
                    for g in (1, 0):
                        for m in range(M):
                            for k in range(KH):
                                nc.tensor.matmul(
                                    psum[g][:, m],
                                    lhsT=Wg_bf[:, g, k, m],
                                    rhs=h_cur[:, k],
                                    start=(k == 0),
                                    stop=(k == KH - 1),
                                )
                        if g == 1:
                            nc.vector.tensor_add(psum[1], psum[1], xzt_cur[:, 1, t])
                            gt = sm_p.tile([128, M, B], dt.float32, tag="gt", name="gt")
                            nc.scalar.activation(gt, psum[1], AF.Tanh)
                            dt_ = sm_p.tile([128, M, B], dt.float32, tag="dt", name="dtl")
                            nc.vector.tensor_sub(dt_, gt, h_cur)
                        else:
                            nc.vector.tensor_add(psum[0], psum[0], xzt_cur[:, 0, t])
                            zt = sm_p.tile([128, M, B], dt.float32, tag="zt", name="zt")
                            nc.scalar.activation(zt, psum[0], AF.Sigmoid)
                            mt = sm_p.tile([128, M, B], dt.float32, tag="mt", name="mt")
                            nc.vector.tensor_mul(mt, zt, dt_)
                            h_new = hbf_p.tile(
                                [128, KH, B], dt.bfloat16, tag="h", name="hn"
                            )
                            nc.vector.tensor_add(h_new, h_cur, mt)
                    # stage fp32 copy of h for output DMA (off critical path)
                    nc.gpsimd.tensor_copy(stage[:, tau], h_new)
                    h_cur = h_new

                    if tau == TG - 1:
                        nc.sync.dma_start(
                            out=outT_d[s - TG + 1 : s + 1].rearrange(
                                "t m p b -> p (t m) b"
                            ),
                            in_=stage.rearrange("p t m b -> p (t m) b"),
                        )

                # leftover precompute groups for next chunk
                for (g_, m_, n_) in pre_groups:
                    emit_pre_group(xbf_next, xzt_next, g_, m_, n_)
                if c + 1 < NCH:
                    xbf_cur, xzt_cur = xbf_next, xzt_next

        for _ in range(reps):
            one_pass()

    return nc


_CACHE = {}


def make_nc(**build_kwargs):
    from concourse import bacc

    nc = bacc.Bacc("TRN2", target_bir_lowering=False, debug=False)
    build(nc, **build_kwargs)
    nc.compile()
    return nc


def _get_nc():
    if "nc" not in _CACHE:
        _CACHE["nc"] = make_nc()
    return _CACHE["nc"]


def kernel(x, Wz, bz, Wh, bh):
    from concourse.bass_utils import run_bass_kernel_spmd

    x = np.ascontiguousarray(np.asarray(x, dtype=np.float32))
    Wz = np.ascontiguousarray(np.asarray(Wz, dtype=np.float32))
    bz = np.ascontiguousarray(np.asarray(bz, dtype=np.float32))
    Wh = np.ascontiguousarray(np.asarray(Wh, dtype=np.float32))
    bh = np.ascontiguousarray(np.asarray(bh, dtype=np.float32))

    nc = _get_nc()
    in_maps = []
    for c in range(NCORES):
        in_maps.append(
            {
                "x": np.ascontiguousarray(x[c * B : (c + 1) * B]),
                "Wz": Wz,
                "bz": bz,
                "Wh": Wh,
                "bh": bh,
            }
        )
    import os
    trace = bool(os.environ.get("GRU_TRACE"))
    res = run_bass_kernel_spmd(
        nc, in_maps, core_ids=list(range(NCORES)),
        trace=trace, trace_cores=[0] if trace else None,
    )
    _CACHE["last_results"] = res
    outputs = np.empty((B_TOT, T, H), dtype=np.float32)
    for c in range(NCORES):
        ot = res.results[c]["outT"]  # (T, M, 128, B)
        outputs[c * B : (c + 1) * B] = (
            ot.transpose(3, 0, 1, 2).reshape(B, T, H)
        )
    h_last = np.ascontiguousarray(outputs[:, -1, :])
    return outputs, h_last


# revision 21
# speedup vs baseline: 1.6741x; 1.6741x over previous
"""MinGRU Trainium2 Bass kernel.

Problem: x (128, 1024, 256) f32, Wz/Wh (768, 512) f32, bz/bh (512,) f32.
    xz = x @ Wz[:256] + bz ; xh = x @ Wh[:256] + bh   (precomputable)
    per step t: z = sigmoid(xz_t + h @ Wz[256:])
                g = tanh   (xh_t + h @ Wh[256:])
                h = (1-z)*h + z*g
Returns (outputs (128, 1024, 512) f32, h_last (128, 512) f32).

Strategy: data-parallel over batch across 8 NeuronCores (16 rows each).
Per core everything lives in "T orientation" (hidden dim on the 128
partitions, batch on the free dim) so the per-step elementwise tail is
cheap ([128, 64] ops) and no transposes are ever needed:

  - recurrent matmuls: stationary = bf16 weight tile W[k,m] (128x128),
    moving = h^T k-slice (128x16), PSUM accumulates a^T m-slice (128x16).
    2 gates x 4 m x 4 k = 32 self-loading matmuls per step.
  - xz/xh are precomputed per 128-step chunk by a GEMM whose moving
    operand is x^T; the precompute matmuls are spread between step
    matmuls so the PE pipeline gaps at step boundaries absorb them.
  - outputs are stored as outT (T, 4, 128, 16) and untransposed on host.

bf16 is used for matmul operands only (weights, h, xz/xh); PSUM
accumulation and the gate arithmetic stay fp32.  Measured error vs the
fp32 reference on hardware: ~3.8e-3 rel l2 / ~7.7e-3 absmax.
"""

import numpy as np

B_TOT, T, I, H = 128, 1024, 256, 512
NCORES = 8
B = B_TOT // NCORES          # 16 batch rows per core
KH = H // 128                # 4 hidden contraction chunks
KI = I // 128                # 2 input contraction chunks
M = H // 128                 # 4 output-hidden chunks
TC = 128                     # time steps per xz/xh precompute chunk
NCH = T // TC                # 8 chunks
TG = 16                      # steps per output staging/DMA group


def build(nc, T=T, TC=TC, TG=TG, reps=1):
    """Emit the full per-core kernel IR into `nc`. Reads DRAM params
    x/Wz/bz/Wh/bh, writes outT. reps>1 repeats the whole pass (identical
    output) -- used only to delta-time one pass on hardware."""
    import concourse.tile as tile
    from concourse import mybir
    from contextlib import ExitStack

    dt = mybir.dt
    AF = mybir.ActivationFunctionType
    NCH = T // TC

    x_d = nc.declare_dram_parameter("x", [B, T, I], dt.float32, isOutput=False)
    Wz_d = nc.declare_dram_parameter("Wz", [I + H, H], dt.float32, isOutput=False)
    bz_d = nc.declare_dram_parameter("bz", [H], dt.float32, isOutput=False)
    Wh_d = nc.declare_dram_parameter("Wh", [I + H, H], dt.float32, isOutput=False)
    bh_d = nc.declare_dram_parameter("bh", [H], dt.float32, isOutput=False)
    outT_d = nc.declare_dram_parameter("outT", [T, M, 128, B], dt.bfloat16, isOutput=True)

    W_d = [Wz_d, Wh_d]
    b_d = [bz_d, bh_d]

    with tile.TileContext(nc) as tc, ExitStack() as ctx:
        const = ctx.enter_context(tc.tile_pool(name="const", bufs=1))
        wtmp_p = ctx.enter_context(tc.tile_pool(name="wtmp", bufs=2))
        xf_p = ctx.enter_context(tc.tile_pool(name="xf", bufs=2))
        xbf_p = ctx.enter_context(tc.tile_pool(name="xbf", bufs=2))
        xzt_p = ctx.enter_context(tc.tile_pool(name="xzt", bufs=2))
        hbf_p = ctx.enter_context(tc.tile_pool(name="hbf", bufs=3))
        sm_p = ctx.enter_context(tc.tile_pool(name="sm", bufs=3))
        ps_step = ctx.enter_context(tc.tile_pool(name="ps_step", bufs=3, space="PSUM"))
        ps_pre = ctx.enter_context(tc.tile_pool(name="ps_pre", bufs=2, space="PSUM"))

        # ---- one-time weight prep: fp32 DRAM -> bf16 SBUF tiles ----
        # recurrent weights: Wg_bf[:, g, k, m, :] is the 128x128 stationary
        # tile for out chunk m, contraction chunk k of gate g.
        Wg_bf = const.tile([128, 2, KH, M, 128], dt.bfloat16, name="Wg_bf")
        # projection weights: Wx_bf[:, g, k, m, :]
        Wx_bf = const.tile([128, 2, KI, M, 128], dt.bfloat16, name="Wx_bf")
        for g in range(2):
            for k in range(KH):
                wt = wtmp_p.tile([128, M, 128], dt.float32, tag="wtmp", name="wt")
                nc.gpsimd.dma_start(
                    out=wt,
                    in_=W_d[g][I + 128 * k : I + 128 * (k + 1), :].rearrange(
                        "p (m c) -> p m c", c=128
                    ),
                )
                nc.gpsimd.tensor_copy(Wg_bf[:, g, k], wt)
            for k in range(KI):
                wt = wtmp_p.tile([128, M, 128], dt.float32, tag="wtmp", name="wt")
                nc.gpsimd.dma_start(
                    out=wt,
                    in_=W_d[g][128 * k : 128 * (k + 1), :].rearrange(
                        "p (m c) -> p m c", c=128
                    ),
                )
                nc.gpsimd.tensor_copy(Wx_bf[:, g, k], wt)
        # biases: bias_sb[:, g*M + m] = b_g[128*m + p]
        bias_sb = const.tile([128, 2 * M], dt.float32, name="bias_sb")
        for g in range(2):
            nc.gpsimd.dma_start(
                out=bias_sb[:, g * M : (g + 1) * M],
                in_=b_d[g].rearrange("(m p) -> p m", p=128),
            )

        # ---- helpers ----
        def emit_x_load(c):
            """DMA chunk c of x (transposed, b-major) and cast to bf16."""
            xf = xf_p.tile([128, KI, B, TC], dt.float32, tag="xf", name="xf")
            xbf = xbf_p.tile([128, KI, B, TC], dt.bfloat16, tag="xbf", name="xbf")
            for k in range(KI):
                for b in range(B):
                    nc.sync.dma_start(
                        out=xf[:, k, b],
                        in_=x_d[b, c * TC : (c + 1) * TC, 128 * k : 128 * (k + 1)]
                        .rearrange("t p -> p t"),
                    )
            nc.gpsimd.tensor_copy(xbf, xf)
            return xbf

        NSUB = (TC * B) // 512      # 512-column sub-tiles of the precompute GEMM
        TSUB = 512 // B             # time steps covered by one sub-tile

        def make_xzt(c):
            return xzt_p.tile([128, 2, TC, M, B], dt.bfloat16, tag="xzt", name="xzt")

        def emit_pre_group(xbf, xzt, g, m, n):
            """One precompute group: xzT[g, ., m, .] for steps of sub-tile n."""
            pp = ps_pre.tile([128, B, TSUB], dt.float32, tag="pp", name="pp")
            for k in range(KI):
                nc.tensor.matmul(
                    pp,
                    lhsT=Wx_bf[:, g, k, m],
                    rhs=xbf[:, k, :, n * TSUB : (n + 1) * TSUB],
                    start=(k == 0),
                    stop=(k == KI - 1),
                )
            nc.scalar.activation(
                xzt[:, g, n * TSUB : (n + 1) * TSUB, m].rearrange("p t b -> p b t"),
                pp,
                AF.Identity,
                bias=bias_sb[:, g * M + m : g * M + m + 1],
            )

        def one_pass():
            # ---- initial state ----
            h_cur = hbf_p.tile([128, KH, B], dt.bfloat16, tag="h", name="h0")
            nc.vector.memset(h_cur, 0.0)

            xbf_cur = emit_x_load(0)
            xzt_cur = make_xzt(0)
            for g in range(2):
                for m in range(M):
                    for n in range(NSUB):
                        emit_pre_group(xbf_cur, xzt_cur, g, m, n)

            # ---- main loop over chunks and steps ----
            xbf_next = xzt_next = None
            for c in range(NCH):
                # prefetch next chunk's x, allocate next xzT
                if c + 1 < NCH:
                    xbf_next = emit_x_load(c + 1)
                    xzt_next = make_xzt(c + 1)
                    pre_groups = [
                        (g, m, n)
                        for g in range(2)
                        for m in range(M)
                        for n in range(NSUB)
                    ]
                    # spread precompute groups across this chunk's steps,
                    # starting a few steps in so the x DMA+cast can land
                    period = max(1, (TC - 8) // len(pre_groups))
                    pre_sched = {8 + i * period for i in range(len(pre_groups))}
                else:
                    pre_groups = []
                    pre_sched = set()

                h_new = None
                for t in range(TC):
                    s = c * TC + t

                    # interleave one precompute group for chunk c+1
                    if pre_groups and t in pre_sched:
                        g_, m_, n_ = pre_groups.pop(0)
                        emit_pre_group(xbf_next, xzt_next, g_, m_, n_)

                    ps_z = ps_step.tile([128, M, B], dt.float32, tag="ps_z", name="ps_z", bufs=2)
                    psh = [
                        ps_step.tile([128, B], dt.float32, tag=f"ps_h{m_}",
                                     name=f"ps_h{m_}", bufs=1)
                        for m_ in range(M)
                    ]
                    # sigmoid is computed as 0.5 + 0.5*tanh(x/2) so every ACT
                    # op uses one table set (no per-step table reloads):
                    #   h' = h + z*(g-h) = v + 0.5*tz*d
                    #   with d = g-h, v = h + 0.5*d, tz = tanh(az/2)
                    #
                    # candidate gate (1) runs k-outer so it can consume the
                    # previous step's h halves as they are produced; its tail
                    # (tanh/d/v) overlaps the update gate's matmuls.  the
                    # update gate (0) runs m-major and its tail is emitted in
                    # halves so h_new[k<2] is ready before h_new[k>=2],
                    # letting the next step's k-outer matmuls start early.
                    gt = sm_p.tile([128, M, B], dt.float32, tag="gt", name="gt")
                    for k in range(KH):
                        for m in range(M):
                            nc.tensor.matmul(
                                psh[m],
                                lhsT=Wg_bf[:, 1, k, m],
                                rhs=h_cur[:, k],
                                start=(k == 0),
                                stop=(k == KH - 1),
                            )
                    for m in range(M):
                        nc.vector.tensor_add(psh[m], psh[m], xzt_cur[:, 1, t, m])
                        nc.scalar.activation(gt[:, m], psh[m], AF.Tanh)
                    dt_ = sm_p.tile([128, M, B], dt.float32, tag="dt", name="dtl")
                    nc.vector.tensor_sub(dt_, gt, h_cur)
                    vt = sm_p.tile([128, M, B], dt.float32, tag="vt", name="vt")
                    nc.vector.scalar_tensor_tensor(
                        vt, dt_, 0.5, h_cur,
                        op0=mybir.AluOpType.mult,
                        op1=mybir.AluOpType.add,
                    )

                    zt = sm_p.tile([128, M, B], dt.float32, tag="zt", name="zt")
                    mt = sm_p.tile([128, M, B], dt.float32, tag="mt", name="mt")
                    h_new = hbf_p.tile([128, KH, B], dt.bfloat16, tag="h", name="hn")
                    for m in range(M):
                        for k in range(KH):
                            nc.tensor.matmul(
                                ps_z[:, m],
                                lhsT=Wg_bf[:, 0, k, m],
                                rhs=h_cur[:, k],
                                start=(k == 0),
                                stop=(k == KH - 1),
                            )
                        if m % 2 == 1:
                            sl = slice(m - 1, m + 1)
                            nc.vector.tensor_add(
                                ps_z[:, sl], ps_z[:, sl], xzt_cur[:, 0, t, sl]
                            )
                            nc.scalar.activation(
                                zt[:, sl], ps_z[:, sl], AF.Tanh, scale=0.5
                            )
                            nc.vector.tensor_mul(mt[:, sl], zt[:, sl], dt_[:, sl])
                            nc.vector.scalar_tensor_tensor(
                                h_new[:, sl], mt[:, sl], 0.5, vt[:, sl],
                                op0=mybir.AluOpType.mult,
                                op1=mybir.AluOpType.add,
                            )
                    # stream h out directly (bf16; host upconverts)
                    nc.sync.dma_start(
                        out=outT_d[s].rearrange("m p b -> p m b"),
                        in_=h_new,
                    )
                    h_cur = h_new

                # leftover precompute groups for next chunk
                for (g_, m_, n_) in pre_groups:
                    emit_pre_group(xbf_next, xzt_next, g_, m_, n_)
                if c + 1 < NCH:
                    xbf_cur, xzt_cur = xbf_next, xzt_next

        for _ in range(reps):
            one_pass()

    return nc


_CACHE = {}


def make_nc(**build_kwargs):
    from concourse import bacc

    nc = bacc.Bacc("TRN2", target_bir_lowering=False, debug=False)
    build(nc, **build_kwargs)
    nc.compile()
    return nc


def _get_nc():
    if "nc" not in _CACHE:
        _CACHE["nc"] = make_nc()
    return _CACHE["nc"]


def kernel(x, Wz, bz, Wh, bh):
    from concourse.bass_utils import run_bass_kernel_spmd

    x = np.ascontiguousarray(np.asarray(x, dtype=np.float32))
    Wz = np.ascontiguousarray(np.asarray(Wz, dtype=np.float32))
    bz = np.ascontiguousarray(np.asarray(bz, dtype=np.float32))
    Wh = np.ascontiguousarray(np.asarray(Wh, dtype=np.float32))
    bh = np.ascontiguousarray(np.asarray(bh, dtype=np.float32))

    nc = _get_nc()
    in_maps = []
    for c in range(NCORES):
        in_maps.append(
            {
                "x": np.ascontiguousarray(x[c * B : (c + 1) * B]),
                "Wz": Wz,
                "bz": bz,
                "Wh": Wh,
                "bh": bh,
            }
        )
    import os
    trace = bool(os.environ.get("GRU_TRACE"))
    res = run_bass_kernel_spmd(
        nc, in_maps, core_ids=list(range(NCORES)),
        trace=trace, trace_cores=[0] if trace else None,
    )
    _CACHE["last_results"] = res
    outputs = np.empty((B_TOT, T, H), dtype=np.float32)
    for c in range(NCORES):
        ot = res.results[c]["outT"]  # (T, M, 128, B) bf16
        outputs[c * B : (c + 1) * B] = (
            ot.transpose(3, 0, 1, 2).reshape(B, T, H).astype(np.float32)
        )
    h_last = np.ascontiguousarray(outputs[:, -1, :])
    return outputs, h_last


# revision 22
# speedup vs baseline: 1.9464x; 1.1627x over previous
"""MinGRU Trainium2 Bass kernel.

Problem: x (128, 1024, 256) f32, Wz/Wh (768, 512) f32, bz/bh (512,) f32.
    xz = x @ Wz[:256] + bz ; xh = x @ Wh[:256] + bh   (precomputable)
    per step t: z = sigmoid(xz_t + h @ Wz[256:])
                g = tanh   (xh_t + h @ Wh[256:])
                h = (1-z)*h + z*g
Returns (outputs (128, 1024, 512) f32, h_last (128, 512) f32).

Strategy: data-parallel over batch across 8 NeuronCores (16 rows each).
Per core everything lives in "T orientation" (hidden dim on the 128
partitions, batch on the free dim) so the per-step elementwise tail is
cheap ([128, 64] ops) and no transposes are ever needed:

  - recurrent matmuls: stationary = bf16 weight tile W[k,m] (128x128),
    moving = h^T k-slice (128x16), PSUM accumulates a^T m-slice (128x16).
    2 gates x 4 m x 4 k = 32 self-loading matmuls per step.
  - xz/xh are precomputed per 128-step chunk by a GEMM whose moving
    operand is x^T; the precompute matmuls are spread between step
    matmuls so the PE pipeline gaps at step boundaries absorb them.
  - outputs are stored as outT (T, 4, 128, 16) and untransposed on host.

bf16 is used for matmul operands only (weights, h, xz/xh); PSUM
accumulation and the gate arithmetic stay fp32.  Measured error vs the
fp32 reference on hardware: ~3.8e-3 rel l2 / ~7.7e-3 absmax.
"""

import numpy as np

B_TOT, T, I, H = 128, 1024, 256, 512
NCORES = 8
B = B_TOT // NCORES          # 16 batch rows per core
KH = H // 128                # 4 hidden contraction chunks
KI = I // 128                # 2 input contraction chunks
M = H // 128                 # 4 output-hidden chunks
TC = 128                     # time steps per xz/xh precompute chunk
NCH = T // TC                # 8 chunks
TG = 16                      # steps per output staging/DMA group


def build(nc, T=T, TC=TC, TG=TG, reps=1):
    """Emit the full per-core kernel IR into `nc`. Reads DRAM params
    x/Wz/bz/Wh/bh, writes outT. reps>1 repeats the whole pass (identical
    output) -- used only to delta-time one pass on hardware."""
    import concourse.tile as tile
    from concourse import mybir
    from contextlib import ExitStack

    dt = mybir.dt
    AF = mybir.ActivationFunctionType
    NCH = T // TC

    x_d = nc.declare_dram_parameter("x", [B, T, I], dt.float32, isOutput=False)
    Wz_d = nc.declare_dram_parameter("Wz", [I + H, H], dt.float32, isOutput=False)
    bz_d = nc.declare_dram_parameter("bz", [H], dt.float32, isOutput=False)
    Wh_d = nc.declare_dram_parameter("Wh", [I + H, H], dt.float32, isOutput=False)
    bh_d = nc.declare_dram_parameter("bh", [H], dt.float32, isOutput=False)
    outT_d = nc.declare_dram_parameter("outT", [T, M, 128, B], dt.bfloat16, isOutput=True)

    W_d = [Wz_d, Wh_d]
    b_d = [bz_d, bh_d]

    with tile.TileContext(nc) as tc, ExitStack() as ctx:
        const = ctx.enter_context(tc.tile_pool(name="const", bufs=1))
        wtmp_p = ctx.enter_context(tc.tile_pool(name="wtmp", bufs=2))
        xf_p = ctx.enter_context(tc.tile_pool(name="xf", bufs=2))
        xbf_p = ctx.enter_context(tc.tile_pool(name="xbf", bufs=2))
        xzt_p = ctx.enter_context(tc.tile_pool(name="xzt", bufs=2))
        hbf_p = ctx.enter_context(tc.tile_pool(name="hbf", bufs=3))
        sm_p = ctx.enter_context(tc.tile_pool(name="sm", bufs=3))
        ps_step = ctx.enter_context(tc.tile_pool(name="ps_step", bufs=3, space="PSUM"))
        ps_pre = ctx.enter_context(tc.tile_pool(name="ps_pre", bufs=2, space="PSUM"))

        # ---- one-time weight prep: fp32 DRAM -> bf16 SBUF tiles ----
        # recurrent weights: Wg_bf[:, g, k, m, :] is the 128x128 stationary
        # tile for out chunk m, contraction chunk k of gate g.
        Wg_bf = const.tile([128, 2, KH, M, 128], dt.bfloat16, name="Wg_bf")
        # projection weights: Wx_bf[:, g, k, m, :]
        Wx_bf = const.tile([128, 2, KI, M, 128], dt.bfloat16, name="Wx_bf")
        for g in range(2):
            for k in range(KH):
                wt = wtmp_p.tile([128, M, 128], dt.float32, tag="wtmp", name="wt")
                nc.gpsimd.dma_start(
                    out=wt,
                    in_=W_d[g][I + 128 * k : I + 128 * (k + 1), :].rearrange(
                        "p (m c) -> p m c", c=128
                    ),
                )
                nc.gpsimd.tensor_copy(Wg_bf[:, g, k], wt)
            for k in range(KI):
                wt = wtmp_p.tile([128, M, 128], dt.float32, tag="wtmp", name="wt")
                nc.gpsimd.dma_start(
                    out=wt,
                    in_=W_d[g][128 * k : 128 * (k + 1), :].rearrange(
                        "p (m c) -> p m c", c=128
                    ),
                )
                nc.gpsimd.tensor_copy(Wx_bf[:, g, k], wt)
        # biases: bias_sb[:, g*M + m] = b_g[128*m + p]
        bias_sb = const.tile([128, 2 * M], dt.float32, name="bias_sb")
        for g in range(2):
            nc.gpsimd.dma_start(
                out=bias_sb[:, g * M : (g + 1) * M],
                in_=b_d[g].rearrange("(m p) -> p m", p=128),
            )

        # ---- helpers ----
        def emit_x_load(c):
            """DMA chunk c of x (transposed, b-major) and cast to bf16."""
            xf = xf_p.tile([128, KI, B, TC], dt.float32, tag="xf", name="xf")
            xbf = xbf_p.tile([128, KI, B, TC], dt.bfloat16, tag="xbf", name="xbf")
            for k in range(KI):
                for b in range(B):
                    nc.sync.dma_start(
                        out=xf[:, k, b],
                        in_=x_d[b, c * TC : (c + 1) * TC, 128 * k : 128 * (k + 1)]
                        .rearrange("t p -> p t"),
                    )
            nc.gpsimd.tensor_copy(xbf, xf)
            return xbf

        NSUB = (TC * B) // 512      # 512-column sub-tiles of the precompute GEMM
        TSUB = 512 // B             # time steps covered by one sub-tile

        def make_xzt(c):
            return xzt_p.tile([128, 2, TC, M, B], dt.bfloat16, tag="xzt", name="xzt")

        def emit_pre_group(xbf, xzt, g, m, n):
            """One precompute group: xzT[g, ., m, .] for steps of sub-tile n."""
            pp = ps_pre.tile([128, B, TSUB], dt.float32, tag="pp", name="pp")
            for k in range(KI):
                nc.tensor.matmul(
                    pp,
                    lhsT=Wx_bf[:, g, k, m],
                    rhs=xbf[:, k, :, n * TSUB : (n + 1) * TSUB],
                    start=(k == 0),
                    stop=(k == KI - 1),
                )
            nc.scalar.activation(
                xzt[:, g, n * TSUB : (n + 1) * TSUB, m].rearrange("p t b -> p b t"),
                pp,
                AF.Identity,
                bias=bias_sb[:, g * M + m : g * M + m + 1],
            )

        def one_pass():
            # ---- initial state ----
            h_cur = hbf_p.tile([128, KH, B], dt.bfloat16, tag="h", name="h0")
            nc.vector.memset(h_cur, 0.0)

            xbf_cur = emit_x_load(0)
            xzt_cur = make_xzt(0)
            for g in range(2):
                for m in range(M):
                    for n in range(NSUB):
                        emit_pre_group(xbf_cur, xzt_cur, g, m, n)

            # ---- main loop over chunks and steps ----
            xbf_next = xzt_next = None
            for c in range(NCH):
                # prefetch next chunk's x, allocate next xzT
                if c + 1 < NCH:
                    xbf_next = emit_x_load(c + 1)
                    xzt_next = make_xzt(c + 1)
                    pre_groups = [
                        (g, m, n)
                        for g in range(2)
                        for m in range(M)
                        for n in range(NSUB)
                    ]
                    # spread precompute groups across this chunk's steps,
                    # starting a few steps in so the x DMA+cast can land
                    period = max(1, (TC - 8) // len(pre_groups))
                    pre_sched = {8 + i * period for i in range(len(pre_groups))}
                else:
                    pre_groups = []
                    pre_sched = set()

                h_new = None
                for t in range(TC):
                    s = c * TC + t

                    # interleave one precompute group for chunk c+1
                    if pre_groups and t in pre_sched:
                        g_, m_, n_ = pre_groups.pop(0)
                        emit_pre_group(xbf_next, xzt_next, g_, m_, n_)

                    psum = [
                        ps_step.tile([128, M, B], dt.float32, tag="ps_z", name="ps_z"),
                        ps_step.tile([128, M, B], dt.float32, tag="ps_h", name="ps_h"),
                    ]
                    # gate 1 (tanh / candidate) first, gate 0 (update) last;
                    # the candidate's chain overlaps the z matmuls.
                    # sigmoid is computed as 0.5 + 0.5*tanh(x/2) so every ACT
                    # op uses one table set (no per-step table reloads):
                    #   h' = h + z*(g-h) = v + 0.5*tz*d
                    #   with d = g-h, v = h + 0.5*d, tz = tanh(az/2)
                    for g in (1, 0):
                        for m in range(M):
                            for k in range(KH):
                                nc.tensor.matmul(
                                    psum[g][:, m],
                                    lhsT=Wg_bf[:, g, k, m],
                                    rhs=h_cur[:, k],
                                    start=(k == 0),
                                    stop=(k == KH - 1),
                                )
                        if g == 1:
                            nc.vector.tensor_add(psum[1], psum[1], xzt_cur[:, 1, t])
                            gt = sm_p.tile([128, M, B], dt.float32, tag="gt", name="gt")
                            nc.scalar.activation(gt, psum[1], AF.Tanh)
                            dt_ = sm_p.tile([128, M, B], dt.float32, tag="dt", name="dtl")
                            nc.vector.tensor_sub(dt_, gt, h_cur)
                            vt = sm_p.tile([128, M, B], dt.float32, tag="vt", name="vt")
                            nc.vector.scalar_tensor_tensor(
                                vt, dt_, 0.5, h_cur,
                                op0=mybir.AluOpType.mult,
                                op1=mybir.AluOpType.add,
                            )
                        else:
                            nc.vector.tensor_add(psum[0], psum[0], xzt_cur[:, 0, t])
                            zt = sm_p.tile([128, M, B], dt.float32, tag="zt", name="zt")
                            nc.scalar.activation(zt, psum[0], AF.Tanh, scale=0.5)
                            mt = sm_p.tile([128, M, B], dt.float32, tag="mt", name="mt")
                            nc.vector.tensor_mul(mt, zt, dt_)
                            h_new = hbf_p.tile(
                                [128, KH, B], dt.bfloat16, tag="h", name="hn"
                            )
                            nc.vector.scalar_tensor_tensor(
                                h_new, mt, 0.5, vt,
                                op0=mybir.AluOpType.mult,
                                op1=mybir.AluOpType.add,
                            )
                    # stream h out directly (bf16; host upconverts)
                    nc.sync.dma_start(
                        out=outT_d[s].rearrange("m p b -> p m b"),
                        in_=h_new,
                    )
                    h_cur = h_new

                # leftover precompute groups for next chunk
                for (g_, m_, n_) in pre_groups:
                    emit_pre_group(xbf_next, xzt_next, g_, m_, n_)
                if c + 1 < NCH:
                    xbf_cur, xzt_cur = xbf_next, xzt_next

        for _ in range(reps):
            one_pass()

    return nc


_CACHE = {}


def make_nc(**build_kwargs):
    from concourse import bacc

    nc = bacc.Bacc("TRN2", target_bir_lowering=False, debug=False)
    build(nc, **build_kwargs)
    nc.compile()
    return nc


def _get_nc():
    if "nc" not in _CACHE:
        _CACHE["nc"] = make_nc()
    return _CACHE["nc"]


def kernel(x, Wz, bz, Wh, bh):
    from concourse.bass_utils import run_bass_kernel_spmd

    x = np.ascontiguousarray(np.asarray(x, dtype=np.float32))
    Wz = np.ascontiguousarray(np.asarray(Wz, dtype=np.float32))
    bz = np.ascontiguousarray(np.asarray(bz, dtype=np.float32))
    Wh = np.ascontiguousarray(np.asarray(Wh, dtype=np.float32))
    bh = np.ascontiguousarray(np.asarray(bh, dtype=np.float32))

    nc = _get_nc()
    in_maps = []
    for c in range(NCORES):
        in_maps.append(
            {
                "x": np.ascontiguousarray(x[c * B : (c + 1) * B]),
                "Wz": Wz,
                "bz": bz,
                "Wh": Wh,
                "bh": bh,
            }
        )
    import os
    trace = bool(os.environ.get("GRU_TRACE"))
    res = run_bass_kernel_spmd(
        nc, in_maps, core_ids=list(range(NCORES)),
        trace=trace, trace_cores=[0] if trace else None,
    )
    _CACHE["last_results"] = res
    outputs = np.empty((B_TOT, T, H), dtype=np.float32)
    for c in range(NCORES):
        ot = res.results[c]["outT"]  # (T, M, 128, B) bf16
        outputs[c * B : (c + 1) * B] = (
            ot.transpose(3, 0, 1, 2).reshape(B, T, H).astype(np.float32)
        )
    h_last = np.ascontiguousarray(outputs[:, -1, :])
    return outputs, h_last
